# revision 40
# baseline (speedup 1.0000x reference)
"""Multi-head causal attention (B=2, S=2048, d_model=1024, H=16) on 8 Trainium2
NeuronCores.

Sharding: core c -> batch b = c // 4, head group g = c % 4 (heads 4g..4g+3).
Data-parallel over the batch, tensor-parallel over heads: each core computes
QKV projections for its 4 heads (column-sliced Wqkv), causal attention for
those heads, and a partial output projection (row-sliced Wo). The host sums
the 4 partial outputs per batch and adds the output bias.

Device dataflow (per core), all matmuls bf16 (fp32 PSUM accumulate):
  Phase 1 (DMA-gated): xT/w stream into SBUF; QKV projections accumulate
  per chunk as the 8 K-tiles arrive. Biases added on the (otherwise idle)
  ACT engine via Identity+bias activations. qT/kT in col-partition layout
  [dim 128, seq]; v natural [seq, head, 65] with a ones column for the
  softmax-denominator row-sum trick (even heads [v|1] -> l at PSUM row 64;
  odd heads [1|v] -> AV lands at PSUM rows 63:128 so the value rows align
  with SBUF partitions 64:128, avoiding any cross-partition move).

  Phase 2 (attention, heads processed in pairs, chunk-pair-serial):
  per (chunk-pair, jt, head): scoresT = kT.T @ qT (PE), exp on ACT
  (scale=1/8 folded, exact causal column trim), diagonal-block triangular
  mask on DVE, AV accumulated into a per-(head,chunk) PSUM bank. Scores for
  the next block are emitted ahead of AV for the current one so PE and ACT
  software-pipeline. At each chunk stop: 1/l on DVE (reciprocal straight
  from PSUM), broadcast across 64 partitions on GPSIMD, PSUM->SBUF copy +
  normalize on DVE into the bf16 values tiles.

  Phase 3 (woven into the second head pair): after head 3 drains chunk ci,
  the output projection for its 4 seq tiles contracts K=128 over each head
  pair and the [128,512] bf16 results stream out, so only the last chunk's
  projection + store trail the attention.
"""

import sys

sys.path.insert(0, "/opt/trn_rl_repo")

import numpy as np

import concourse.bass as bass
import concourse.mybir as mybir
import concourse.tile as tile
from concourse.bass_utils import run_bass_kernel_spmd

F32 = mybir.dt.float32
F32R = mybir.dt.float32r
BF16 = mybir.dt.bfloat16

B, S, D = 2, 2048, 1024
H_TOT = 16
HD = 64
H_PER_CORE = 4
N_CORES = 8
SCALE = 1.0 / np.sqrt(HD)

ST = S // 128   # 16 sequence tiles of 128
NCH = S // 512  # 4 query chunks of 512
LAST_CI = NCH - 1  # pair-2's last-processed chunk (ascending order)


def _split_multi_waits(nc):
    """This container's walrus rejects >1 sem wait per instruction. Move
    extra waits onto fresh single-wait NOPs on the same engine, inserted
    immediately before the instruction (same-engine streams are in-order,
    so semantics are unchanged)."""
    n = 0
    for func in nc.m.functions:
        for bb in func.blocks:
            i = 0
            while i < len(bb.instructions):
                ins = bb.instructions[i]
                si = ins.sync_info
                if si is not None and si.on_wait and len(si.on_wait) > 1:
                    waits = list(si.on_wait)
                    si.on_wait = [waits[-1]]
                    eng = nc.engines[ins.engine]
                    nops = []
                    for w in waits[:-1]:
                        ni = eng.nop(nofuse=True, hint="wait_split").ins
                        if ni.sync_info is None:
                            ni.sync_info = mybir.SyncInfo(on_wait=[w], on_update=[])
                        else:
                            ni.sync_info.on_wait = [w]
                        nops.append(ni)
                    for ni in nops:
                        for f2 in nc.m.functions:
                            for bb2 in f2.blocks:
                                if ni in bb2.instructions:
                                    bb2.instructions.remove(ni)
                    for k, ni in enumerate(nops):
                        bb.instructions.insert(i + k, ni)
                    i += len(nops)
                    n += len(nops)
                i += 1
    return n


def _dram_row_bcast(handle, offset_elems, width, parts):
    """AP that broadcasts a DRAM row of `width` elems across `parts` partitions."""
    return bass.AP(tensor=handle, offset=offset_elems, ap=[[0, parts], [1, width]])


def build_bass(reps=1):
    """reps>1 unrolls the whole kernel body `reps` times into one NEFF —
    used only for timing: the repeated executions serialize on-device, so
    the marginal time between a reps=R and reps=1 NEFF isolates true
    device time from the (large, variable) per-dispatch overhead."""
    nc = bass.Bass()

    xT = nc.dram_tensor("xT", [D, S], BF16, kind="ExternalInput")
    w = nc.dram_tensor("w", [D, 768], BF16, kind="ExternalInput")
    bias_qk = nc.dram_tensor("bias_qk", [128, 4], F32, kind="ExternalInput")
    bias_v = nc.dram_tensor("bias_v", [256], F32, kind="ExternalInput")
    wo = nc.dram_tensor("wo", [256, D], F32R, kind="ExternalInput")
    tri = nc.dram_tensor("tri", [128, 128], BF16, kind="ExternalInput")
    out = nc.dram_tensor("out", [S, D], BF16, kind="ExternalOutput")

    # Per-queue DMA bandwidth is low in this environment; round-robin bulk
    # transfers across all three DMA-capable queues.
    _dma_engines = [nc.sync, nc.scalar, nc.gpsimd]
    _dma_i = [0]

    def dma_rr(out_ap, in_ap):
        e = _dma_engines[_dma_i[0] % len(_dma_engines)]
        _dma_i[0] += 1
        return e.dma_start(out_ap, in_ap)

    # Store queues: ACT's queue carries DMA-trigger sem waits that would
    # stall the exp stream, so stores ride sync+gpsimd until the last chunk.
    _dma2_engines = [nc.sync, nc.gpsimd]
    _dma2_i = [0]

    def dma_rr2(out_ap, in_ap, with_act=False):
        engines = _dma_engines if with_act else _dma2_engines
        idx = _dma_i if with_act else _dma2_i
        e = engines[idx[0] % len(engines)]
        idx[0] += 1
        return e.dma_start(out_ap, in_ap)

    with tile.TileContext(nc) as tc:
      for _rep in range(reps):
        with (
            tc.tile_pool(name="consts", bufs=1) as consts,
            tc.tile_pool(name="qkT_p", bufs=1) as qkT_p,
            tc.tile_pool(name="v_p", bufs=1) as v_p,
            tc.tile_pool(name="values_p", bufs=1) as values_p,
            tc.tile_pool(name="pT_p", bufs=6) as pT_p,
            tc.tile_pool(name="lrow_p", bufs=4) as lrow_p,
            tc.tile_pool(name="vtmp_p", bufs=4) as vtmp_p,
            tc.tile_pool(name="out_p", bufs=6) as out_p,
        ):
            # ---- constants ----
            bias_qk_sb = consts.tile([128, 4], F32)
            vbias_bc = consts.tile([128, 256], F32)
            wo_sb = [consts.tile([128, D], F32R, name=f"wo{i}") for i in range(2)]
            tri_sb = consts.tile([128, 128], BF16)
            ones_hi = consts.tile([128, 64], F32R)
            nc.vector.memset(ones_hi[64:65, :].bitcast(F32), 1.0)

            # persistent activation tensors
            qkT = [qkT_p.tile([128, S], BF16, name=f"qkT{mt}") for mt in range(4)]
            v_sb = [v_p.tile([128, H_PER_CORE, 65], BF16, name=f"v{st}") for st in range(ST)]
            values = [values_p.tile([128, S], F32R, name=f"vals{hp}") for hp in range(2)]

            # ---- phase 1: load x/W, QKV projections for heads 0/1 + v ----
            # q23/k23 (mt 1,3) are deferred and woven into head-pair 1's
            # attention stream, where ACT is the bottleneck and PE has slack.
            xw_p = tc.alloc_tile_pool(name="xw_p", bufs=1)
            xt = [xw_p.tile([128, S], BF16, name=f"xt{k}") for k in range(8)]
            wt = [xw_p.tile([128, 768], BF16, name=f"wt{k}") for k in range(8)]
            with (
                tc.tile_pool(name="qkv_ps", bufs=2, space="PSUM") as qkv_ps,
                tc.tile_pool(name="vps", bufs=2, space="PSUM") as vps,
            ):
                for k in range(8):
                    dma_rr(xt[k][:], xT[128 * k : 128 * (k + 1), :])
                    dma_rr(wt[k][:], w[128 * k : 128 * (k + 1), :])
                # constants ride the queues behind the critical x/W stream
                nc.sync.dma_start(bias_qk_sb[:], bias_qk[:])
                nc.scalar.dma_start(vbias_bc[:], _dram_row_bcast(bias_v, 0, 256, 128))
                nc.gpsimd.dma_start(tri_sb[:], tri[:])
                for i in range(2):
                    dma_rr(wo_sb[i][:], wo[128 * i : 128 * (i + 1), :])

                # qT/kT: psum[col 128, s 1024] accumulated over 8 K-tiles,
                # bias added on ACT (idle during the load phase).
                # mt 0/1 = q heads 01/23, mt 2/3 = k heads 01/23.
                for mt in (0, 2):
                    for ch in range(NCH):
                        pq = qkv_ps.tile([128, 512], F32, name="pq")
                        for k in range(8):
                            nc.tensor.matmul(
                                pq[:],
                                wt[k][:, 128 * mt : 128 * (mt + 1)],
                                xt[k][:, 512 * ch : 512 * (ch + 1)],
                                start=(k == 0),
                                stop=(k == 7),
                            )
                        nc.scalar.activation(
                            qkT[mt][:, 512 * ch : 512 * (ch + 1)],
                            pq[:],
                            mybir.ActivationFunctionType.Identity,
                            bias=bias_qk_sb[:, mt : mt + 1],
                        )
                # v natural: psum[s 128, vcol 256] over 8 K-tiles; bias via
                # broadcast TT-add during copy-out; ones column via memset.
                for st in range(ST):
                    pv = vps.tile([128, 256], F32, name="pv")
                    for k in range(8):
                        nc.tensor.matmul(
                            pv[:],
                            xt[k][:, 128 * st : 128 * (st + 1)],
                            wt[k][:, 512:768],
                            start=(k == 0),
                            stop=(k == 7),
                        )
                    nc.vector.memset(v_sb[st][:, :, 64:65], 1.0)
                    nc.vector.tensor_tensor(
                        v_sb[st][:, :, 0:64],
                        pv[:].rearrange("p (h d) -> p h d", h=H_PER_CORE),
                        vbias_bc[:].rearrange("p (h d) -> p h d", h=H_PER_CORE),
                        mybir.AluOpType.add,
                    )

            # ---- phase 2: attention, head pairs ----
            def drain_a(h, ci, av):
                """Chunk ci of head h just stopped accumulating in av (l in
                row 64): kick off 1/l on DVE. The PE broadcast runs a jt
                slot later (drain_b) so it never waits on the reciprocal."""
                lrow = lrow_p.tile([128, 512], F32R, name="lrow")
                with nc.allow_low_precision(reason="fp32r 1/l: fp22 rounding on a pure scale"):
                    nc.vector.reciprocal(lrow[64:65, :], av[64:65, :])
                return lrow

            def drain_b(h, ci, av, lrow, store_act=False, act_copy=False):
                """Copy the raw AV out of PSUM, then broadcast 1/l across 64
                partitions via a K=1 PE matmul aimed back INTO the just-read
                av rows (no extra PSUM bank), and normalize into the F32R
                values tiles. Odd heads' value rows must land on partitions
                64:128, which no engine can reach from av rows 0:64, so they
                normalize into a scratch tile and hop via SBUF->SBUF DMA."""
                hp = h // 2
                odd = h % 2
                if odd:
                    vslice = vtmp_p.tile([64, 512], F32R, name="vtmp")[:]
                else:
                    vslice = values[hp][0:64, 512 * ci : 512 * (ci + 1)]
                if act_copy:
                    nc.scalar.copy(vslice, av[0:64, :])
                else:
                    nc.vector.tensor_copy(vslice, av[0:64, :])
                nc.tensor.matmul(
                    av[0:64, :],
                    ones_hi[64:65, :],
                    lrow[64:65, :],
                    start=True,
                    stop=True,
                )
                nc.vector.tensor_tensor(
                    vslice, vslice, av[0:64, :].bitcast(F32R), mybir.AluOpType.mult
                )
                if odd:
                    dma_rr2(
                        values[hp][64:128, 512 * ci : 512 * (ci + 1)],
                        vslice,
                        with_act=store_act,
                    )

            def make_proj_task(st, nh, ci, copy_i, po_ps):
                """One projection+store unit for seq tile st, out half nh.
                Woven into pair-2's PE stream one per jt slot so the
                PSUM->SBUF copy load spreads instead of bursting at chunk
                boundaries. Copies alternate DVE/ACT; the last chunk's all
                ride ACT (its exp stream is finished by then)."""
                def task():
                    po = po_ps.tile([128, 512], F32, name="po")
                    for hp in range(2):
                        nc.tensor.matmul(
                            po[:],
                            values[hp][:, 128 * st : 128 * (st + 1)],
                            wo_sb[hp][:, 512 * nh : 512 * (nh + 1)],
                            start=(hp == 0),
                            stop=(hp == 1),
                        )
                    o_sb = out_p.tile([128, 512], BF16, name="o_sb")
                    if ci == LAST_CI and copy_i[0] % 2 == 0:
                        nc.scalar.copy(o_sb[:], po[:])
                    else:
                        nc.vector.tensor_copy(o_sb[:], po[:])
                    copy_i[0] += 1
                    dma_rr2(
                        out[128 * st : 128 * (st + 1), 512 * nh : 512 * (nh + 1)],
                        o_sb[:],
                        with_act=(ci == LAST_CI),
                    )
                return task

            def attn_scores(h, jt, p_lo, p_hi, sT, pT, start_col):
                """Scores+exp(+mask) for head h, key tile jt, query columns
                [max(j0,p_lo), p_hi). Matmuls split at 512-col PSUM bank
                boundaries; ONE exp op covers the whole span (the per-op
                PSUM access penalty on ACT is the exp overhead that
                matters). sT cols map to [p_lo,...], pT to [start_col,...]."""
                hp, hr = h // 2, 64 * (h % 2)
                j0 = 128 * jt
                s0 = max(j0, p_lo)
                q_t = qkT[hp]
                k_t = qkT[2 + hp]
                c = s0
                while c < p_hi:
                    c1 = min(p_hi, (c // 512 + 1) * 512)
                    nc.tensor.matmul(
                        sT[:, c - p_lo : c1 - p_lo],
                        k_t[hr : hr + 64, j0 : j0 + 128],
                        q_t[hr : hr + 64, c:c1],
                        start=True,
                        stop=True,
                    )
                    c = c1
                nc.scalar.activation(
                    pT[:, s0 - start_col : p_hi - start_col],
                    sT[:, s0 - p_lo : p_hi - p_lo],
                    mybir.ActivationFunctionType.Exp,
                    scale=float(SCALE),
                )
                if p_lo <= j0 < p_hi:
                    # diagonal block: zero the i < j half
                    nc.vector.tensor_tensor(
                        pT[:, j0 - start_col : j0 - start_col + 128],
                        pT[:, j0 - start_col : j0 - start_col + 128],
                        tri_sb[:],
                        mybir.AluOpType.mult,
                    )

            def attn_av(h, jt, ci, pT, start_col, av):
                """AV accumulate for chunk ci of head h from pT."""
                j0 = 128 * jt
                g0 = max(512 * ci, j0)
                g1 = 512 * (ci + 1)
                nc.tensor.matmul(
                    av[0:65, g0 - 512 * ci : 512],
                    v_sb[jt][:, h, :],
                    pT[:, g0 - start_col : g1 - start_col],
                    start=(jt == 0),
                    stop=(jt == 4 * ci + 3),
                )

            def head_pair(heads, groups, sT_pool, av_pool, tasks, on_drain, rate):
                """Attention for a head pair, chunk-group-serial. Each (h, jt)
                gets one sT block spanning the whole group (<=1024 cols -> one
                exp op). Diagonal-block AVs are deferred one jt slot so the
                exp->tri-mask->AV chain never stalls the in-order PE stream;
                drains are split the same way (reciprocal one slot before the
                broadcast+normalize). `tasks` is a deque of callables woven
                into the PE stream every `rate` jt slots (QKV for the second
                pair / projection+store units); on_drain may append to it."""
                span = 512 * len(groups[0])
                copy_i = [0]
                slot = 0
                for group in groups:
                    base = 512 * group[0]
                    g_hi = 512 * (group[-1] + 1)
                    av = {}
                    for h in heads:
                        for ci in group:
                            av[(h, ci)] = av_pool.tile(
                                [128, 512], F32, name=f"av{h}_{ci % 2}", tag="av"
                            )
                    pend = []
                    pend_dr = []
                    for jt in range(4 * group[-1] + 4):
                        j0 = 128 * jt
                        start_col = max(j0, base)
                        new_pend = []
                        for h in heads:
                            pT = pT_p.tile([128, 1024], BF16, name="pT")
                            sT = sT_pool.tile([128, span], F32, name="sT")
                            attn_scores(h, jt, base, g_hi, sT, pT, start_col)
                            for ci in group:
                                if 512 * (ci + 1) <= j0:
                                    continue
                                item = (h, jt, ci, pT, start_col)
                                if 512 * ci <= j0:
                                    new_pend.append(item)  # diagonal: defer
                                else:
                                    attn_av(*item, av[(h, ci)])
                        new_pend_dr = []
                        for h, pjt, ci, pT, sc in pend:
                            attn_av(h, pjt, ci, pT, sc, av[(h, ci)])
                            if pjt == 4 * ci + 3:
                                lrow = drain_a(h, ci, av[(h, ci)])
                                new_pend_dr.append((h, ci, av[(h, ci)], lrow))
                        for h, ci, avt, lrow in pend_dr:
                            on_drain(h, ci, avt, lrow, copy_i)
                        pend = new_pend
                        pend_dr = new_pend_dr
                        slot += 1
                        if tasks and slot % rate == 0:
                            tasks.popleft()()
                    for h, pjt, ci, pT, sc in pend:
                        attn_av(h, pjt, ci, pT, sc, av[(h, ci)])
                        if pjt == 4 * ci + 3:
                            lrow = drain_a(h, ci, av[(h, ci)])
                            pend_dr.append((h, ci, av[(h, ci)], lrow))
                    for h, ci, avt, lrow in pend_dr:
                        on_drain(h, ci, avt, lrow, copy_i)
                while tasks:
                    tasks.popleft()()

            # ---- head pair 1 (heads 0,1), q23/k23 QKV woven into PE slack ----
            with (
                tc.tile_pool(name="sT1_ps", bufs=2, space="PSUM") as sT1_ps,
                tc.tile_pool(name="av1_ps", bufs=4, space="PSUM") as av1_ps,
            ):
                def make_qkv_task(mt, ch):
                    def task():
                        # shares the sT ring (same name -> same slots); the
                        # two 512-wide matmul groups respect PSUM bank
                        # boundaries, one bias-add covers the 1024 block.
                        pq = sT1_ps.tile([128, 1024], F32, name="sT")
                        for half in range(2):
                            for k in range(8):
                                nc.tensor.matmul(
                                    pq[:, 512 * half : 512 * (half + 1)],
                                    wt[k][:, 128 * mt : 128 * (mt + 1)],
                                    xt[k][
                                        :,
                                        1024 * ch + 512 * half : 1024 * ch + 512 * (half + 1),
                                    ],
                                    start=(k == 0),
                                    stop=(k == 7),
                                )
                        nc.vector.tensor_scalar(
                            qkT[mt][:, 1024 * ch : 1024 * (ch + 1)],
                            pq[:],
                            bias_qk_sb[:, mt : mt + 1],
                            None,
                            mybir.AluOpType.add,
                        )
                    return task

                from collections import deque

                qkv_tasks = deque(
                    make_qkv_task(mt, ch) for mt in (1, 3) for ch in range(2)
                )

                def drain1(h, ci, av, lrow, copy_i):
                    # pair-1: ACT has slack, DVE is the drain hotspot
                    drain_b(h, ci, av, lrow, act_copy=True)

                head_pair(
                    (0, 1), [(0, 1), (2, 3)], sT1_ps, av1_ps, qkv_tasks, drain1, rate=3
                )
            xw_p.release()

            # ---- head pair 2 (heads 3,2): chunk-single (av needs only 2
            # banks, freeing sT pipeline depth + projection banks).
            # Projection + store woven in after each chunk fully drains; h2
            # (direct values write, no DMA hop) drains last so the
            # projection isn't gated on an SBUF->SBUF DMA. ----
            with (
                tc.tile_pool(name="av2_ps", bufs=2, space="PSUM") as av2_ps,
                tc.tile_pool(name="sT2_ps", bufs=4, space="PSUM") as sT2_ps,
                tc.tile_pool(name="po_ps", bufs=2, space="PSUM") as po_ps,
            ):
                from collections import deque

                proj_tasks = deque()

                def drain2(h, ci, av, lrow, copy_i):
                    drain_b(h, ci, av, lrow, store_act=(ci == LAST_CI))
                    if h == 2:
                        for st in range(4 * ci, 4 * ci + 4):
                            for nh in range(2):
                                proj_tasks.append(
                                    make_proj_task(st, nh, ci, copy_i, po_ps)
                                )

                head_pair(
                    (3, 2),
                    [(0,), (1,), (2,), (3,)],
                    sT2_ps,
                    av2_ps,
                    proj_tasks,
                    drain2,
                    rate=1,
                )

    _split_multi_waits(nc)
    return nc


_NC_CACHE = {}


def _get_nc(reps=1):
    if reps not in _NC_CACHE:
        _NC_CACHE[reps] = build_bass(reps)
    return _NC_CACHE[reps]


def make_in_maps(x, mask, Wqkv, bqkv, Wo, bo):
    x = np.asarray(x, dtype=np.float32)
    Wqkv = np.asarray(Wqkv, dtype=np.float32)
    bqkv = np.asarray(bqkv, dtype=np.float32)
    Wo = np.asarray(Wo, dtype=np.float32)

    import ml_dtypes

    xT = [np.ascontiguousarray(x[b].T).astype(ml_dtypes.bfloat16) for b in range(B)]
    tri = (np.arange(128)[None, :] >= np.arange(128)[:, None]).astype(ml_dtypes.bfloat16)

    in_maps = []
    for c in range(N_CORES):
        b, g = c // 4, c % 4
        heads = [4 * g + h for h in range(H_PER_CORE)]
        # Wqkv columns are per-head interleaved: head H -> q cols
        # 192H..192H+64, k cols 192H+64.., v cols 192H+128..
        iq = np.concatenate([np.arange(192 * H, 192 * H + 64) for H in heads])
        ik = np.concatenate([np.arange(192 * H + 64, 192 * H + 128) for H in heads])
        iv = np.concatenate([np.arange(192 * H + 128, 192 * H + 192) for H in heads])
        w_c = np.ascontiguousarray(
            np.concatenate([Wqkv[:, iq], Wqkv[:, ik], Wqkv[:, iv]], axis=1)
        ).astype(ml_dtypes.bfloat16)
        bias_qk = np.stack(
            [bqkv[iq[:128]], bqkv[iq[128:]], bqkv[ik[:128]], bqkv[ik[128:]]],
            axis=1,
        ).astype(np.float32)
        bias_v = np.ascontiguousarray(bqkv[iv])
        wo_c = np.ascontiguousarray(Wo[256 * g : 256 * (g + 1), :])
        in_maps.append(
            {
                "xT": xT[b],
                "w": w_c,
                "bias_qk": bias_qk,
                "bias_v": bias_v,
                "wo": wo_c,
                "tri": tri,
            }
        )
    return in_maps


def _make_chain_runner(nc, in_maps):
    """jit'd donated-chain runner for one bass module; returns run(reps_calls)
    -> wall seconds."""
    import time

    import jax
    from jax.sharding import Mesh, PartitionSpec
    from jax.experimental.shard_map import shard_map
    from concourse import bass2jax
    from concourse.bass2jax import _bass_exec_p

    partition_name = nc.partition_id_tensor.name if nc.partition_id_tensor else None
    in_names, out_names, out_avals, zero_shapes = [], [], [], []
    for alloc in nc.m.functions[0].allocations:
        if not isinstance(alloc, mybir.MemoryLocationSet):
            continue
        name = alloc.memorylocations[0].name
        if alloc.kind == "ExternalInput":
            if name != partition_name:
                in_names.append(name)
        elif alloc.kind == "ExternalOutput":
            out_names.append(name)
            shape = tuple(alloc.tensor_shape)
            dtype = mybir.dt.np(alloc.dtype)
            out_avals.append(jax.core.ShapedArray(shape, dtype))
            zero_shapes.append((shape, dtype))
    n_params = len(in_names)
    n_outs = len(out_avals)
    all_in_names = list(in_names) + list(out_names)
    if partition_name is not None:
        all_in_names.append(partition_name)

    def _body(*args):
        operands = list(args)
        if partition_name is not None:
            operands.append(bass2jax.partition_id_tensor())
        outs = _bass_exec_p.bind(
            *operands,
            out_avals=tuple(out_avals),
            in_names=tuple(all_in_names),
            out_names=tuple(out_names),
            lowering_input_output_aliases=(),
            sim_require_finite=True,
            sim_require_nnan=True,
            nc=nc,
        )
        return tuple(outs)

    devices = jax.devices()[:N_CORES]
    mesh = Mesh(np.asarray(devices), ("core",))
    donate = tuple(range(n_params, n_params + n_outs))
    sharded = jax.jit(
        shard_map(
            _body,
            mesh=mesh,
            in_specs=(PartitionSpec("core"),) * (n_params + n_outs),
            out_specs=(PartitionSpec("core"),) * n_outs,
            check_rep=False,
        ),
        donate_argnums=donate,
        keep_unused=True,
    )
    sharding = jax.sharding.NamedSharding(mesh, PartitionSpec("core"))
    concat_in = [
        np.concatenate([np.asarray(in_maps[c][in_names[i]]) for c in range(N_CORES)], axis=0)
        for i in range(n_params)
    ]
    dev_in = [jax.device_put(a, sharding) for a in concat_in]

    def run(calls):
        outs = [
            jax.device_put(np.zeros((N_CORES * s[0], *s[1:]), dt), sharding)
            for (s, dt) in zero_shapes
        ]
        for z in outs:
            z.block_until_ready()
        t0 = time.perf_counter()
        for _ in range(calls):
            outs = sharded(*dev_in, *outs)
        for o in outs:
            o.block_until_ready()
        return time.perf_counter() - t0

    return run


def bench(x, mask, Wqkv, bqkv, Wo, bo, iters=20, reps=5, calls=49):
    """Device-time measurement that survives dispatch congestion: two NEFFs,
    one containing the kernel body once and one containing it `reps` times
    (the unrolled copies serialize on-device). Adjacent same-length donated
    chains of each are timed back-to-back, so per-call dispatch overhead and
    the fixed RPC cost are common-mode and cancel in the difference:
    device_ns = (t_R - t_1) / (calls * (reps - 1)). Returns (median, info)."""
    from concourse.bass2jax import install_neuronx_cc_hook

    install_neuronx_cc_hook()
    in_maps = make_in_maps(x, mask, Wqkv, bqkv, Wo, bo)
    run1 = _make_chain_runner(_get_nc(1), in_maps)
    runR = _make_chain_runner(_get_nc(reps), in_maps)
    run1(1)
    runR(1)  # warm both NEFFs
    ests = []
    raw = []
    for _ in range(iters):
        t1 = run1(calls)
        tR = runR(calls)
        raw.append((t1, tR))
        ests.append((tR - t1) / (calls * (reps - 1)) * 1e9)
    s = sorted(ests)
    med = s[len(s) // 2]
    best = med
    info = {
        "ests_ns": [round(e) for e in ests],
        "marginal_best": min(s),
        "marginal_med": med,
        "lo": [t1 * 1e9 for t1, tR in raw],
        "hi": [tR * 1e9 for t1, tR in raw],
    }
    return best, info


def kernel(x, mask, Wqkv, bqkv, Wo, bo, _trace=False):
    nc = _get_nc()
    in_maps = make_in_maps(x, mask, Wqkv, bqkv, Wo, bo)
    res = run_bass_kernel_spmd(nc, in_maps, core_ids=list(range(N_CORES)), trace=_trace)
    partials = [np.asarray(r["out"], dtype=np.float32) for r in res.results]
    bo = np.asarray(bo, dtype=np.float32)
    out = np.empty((B, S, D), dtype=np.float32)
    for b in range(B):
        out[b] = partials[4 * b] + partials[4 * b + 1] + partials[4 * b + 2] + partials[4 * b + 3] + bo
    if _trace:
        return out, res
    return out
